# revision 1
# baseline (speedup 1.0000x reference)
"""Spectral pooling (FFT2 -> crop low freqs -> IFFT2) as dense DFT matmuls on TRN2.

Input  x: (32, 256, 64, 64) fp32  -- channels 0:128 real part, 128:256 imag part
Output y: (32, 256, 32, 32) fp32

Math: per complex image X (64x64), Y = A @ X @ A.T with
  A = sqrt(1/(64*32)) * IDFT32 @ Crop @ DFT64   (32x64 complex)
Sharding: batch dim across 8 cores (4 batches/core), no communication.

On-chip scheme (bf16 matmuls, K=M=128, data stationary, DFT matrices moving):
  stage 1: lhsT = [[Xr_c, Xr_c2], [Xr_c1, Xr_c3]] block matrix (a "quad" of 4
           complex channels), rhs = block-structured [Ar.T/Ai.T] constants
           -> psum1 = [Pr_c|Pr_c1|Pi_c|Pi_c1 ; Pr_c2|Pr_c3|Pi_c2|Pi_c3].T-ish
  stage 2: lhsT = psum1 column halves, rhs = block-diag constants -> Y quad
  2 matmuls per stage per quad (real+imag accumulate in PSUM).
  fp32->bf16 input cast happens inside the SWDGE load DMA; outputs restored to
  fp32 by the PSUM->SBUF copies (DVE stage-1 cast, ACT stage-2).
"""

import math

import numpy as np

from concourse import bass, mybir
from concourse.bass_utils import run_bass_kernel_spmd
from concourse.tile import TileContext

N_CORES = 8
B_FULL, C2, H, W = 32, 256, 64, 64
HP, WP = 32, 32
BPC = B_FULL // N_CORES  # batches per core

F32 = mybir.dt.float32
BF16 = mybir.dt.bfloat16


def _split_multi_waits(nc):
    """This walrus build rejects instructions carrying more than one semaphore
    wait. Hoist extra waits onto same-engine NOPs inserted just before the
    instruction (engine queues execute in order, so blocking is equivalent)."""
    n_split = 0
    for f in nc.m.functions:
        for bb in f.blocks:
            insts = bb.instructions
            out = []
            for inst in insts:
                si = inst.sync_info
                waits = list(si.on_wait) if si and si.on_wait else []
                if len(waits) > 1:
                    si.on_wait = waits[-1:]
                    for w in waits[:-1]:
                        nop = mybir.InstNoOp(
                            name=nc.get_next_instruction_name(),
                            ins=[],
                            outs=[],
                            engine=inst.engine,
                            sync_info=mybir.SyncInfo(on_wait=[w], on_update=[]),
                        )
                        out.append(nop)
                        n_split += 1
                out.append(inst)
            if len(out) != len(insts):
                insts[:] = out
    return n_split


def _dft_constants():
    """[4, 128, 128] fp32: stage-1 (D1r, D1i) and stage-2 (D2r, D2i) moving
    operands."""
    topf = int(math.ceil(H * 0.5 / 2))  # 16
    midf = H // 2 + topf  # 48
    F = np.exp(-2j * np.pi * np.outer(np.arange(H), np.arange(H)) / H)
    G = np.exp(2j * np.pi * np.outer(np.arange(HP), np.arange(HP)) / HP)
    keep = list(range(topf)) + list(range(midf, H))
    S = np.zeros((HP, H))
    S[np.arange(HP), keep] = 1
    A = (G @ S @ F) / np.sqrt(H * W * HP * WP) ** 0.5
    ArT = A.real.astype(np.float32).T  # [64, 32]
    AiT = A.imag.astype(np.float32).T

    D1r = np.zeros((128, 128), np.float32)
    D1i = np.zeros((128, 128), np.float32)
    D1r[:64, 0:32] = ArT
    D1r[64:, 32:64] = ArT
    D1r[:64, 64:96] = AiT
    D1r[64:, 96:128] = AiT
    D1i[:64, 0:32] = -AiT
    D1i[64:, 32:64] = -AiT
    D1i[:64, 64:96] = ArT
    D1i[64:, 96:128] = ArT

    C2r = np.concatenate([ArT, AiT], axis=1)  # [64, 64]
    C2i = np.concatenate([-AiT, ArT], axis=1)
    D2r = np.zeros((128, 128), np.float32)
    D2i = np.zeros((128, 128), np.float32)
    D2r[:64, :64] = C2r
    D2r[64:, 64:] = C2r
    D2i[:64, :64] = C2i
    D2i[64:, 64:] = C2i
    return np.stack([D1r, D1i, D2r, D2i])


def build_program(reps: int = 1, split_waits: bool = True,
                  sp_loads: bool = False, sp_stores: bool = False,
                  gp_stores: bool = False, load_mode: str = "l1",
                  probe_contig_loads: bool = False,
                  probe_contig_stores: bool = False,
                  probe_no_compute: bool = False,
                  probe_no_dma: bool = False,
                  no_tilepos: bool = False,
                  deep_bufs: bool = False,
                  interleave_qp: bool = False):
    """reps > 1 unrolls the whole pipeline in-NEFF over the same data so the
    marginal cost per rep can be measured without the ~65ms axon dispatch
    overhead."""
    nc = bass.Bass("TRN2", target_bir_lowering=False, debug=False)
    x = nc.dram_tensor("x", [BPC, C2, H, W], F32, kind="ExternalInput").ap()
    dm = nc.dram_tensor("dmats", [4, 128, 128], F32, kind="ExternalInput").ap()
    y = nc.dram_tensor("y", [BPC, C2, HP, WP], F32, kind="ExternalOutput").ap()

    with TileContext(nc) as tc:
        with (
            tc.tile_pool(name="consts", bufs=1) as cpool,
            tc.tile_pool(name="inp", bufs=3 if deep_bufs else 2) as ipool,
            tc.tile_pool(name="sb1", bufs=6 if deep_bufs else 4) as s1pool,
            tc.tile_pool(name="sbout", bufs=3 if deep_bufs else 2) as opool,
            tc.tile_pool(name="ps1", bufs=4, space="PSUM") as p1pool,
            tc.tile_pool(name="ps2", bufs=4 if deep_bufs else 2,
                         space="PSUM") as p2pool,
        ):
            dmf = cpool.tile([128, 512], F32, tag="dmf")
            dmb = cpool.tile([128, 512], BF16, tag="dmb")
            for k in range(4):
                nc.sync.dma_start(out=dmf[:, 128 * k : 128 * (k + 1)], in_=dm[k])
            nc.vector.tensor_copy(out=dmb, in_=dmf)
            d1rb = dmb[:, 0:128]
            d1ib = dmb[:, 128:256]
            d2rb = dmb[:, 256:384]
            d2ib = dmb[:, 384:512]

            for b in [b for _ in range(reps) for b in range(BPC)]:
                # SBUF input layout: partitions = (channel parity, h); free =
                # (slot, w) where slot s holds pair 4*(s//4) + {0,2,1,3}[s%4]
                # so each quad (o, qp) is a contiguous 128-col lhsT slice.
                in_r = ipool.tile([128, (C2 // 4) * W], BF16, tag="in_r")
                in_i = ipool.tile([128, (C2 // 4) * W], BF16, tag="in_i")
                # channel = 8o + 4bb + 2rr + two ; pair = 4o + 2bb + rr
                # slot 4o + 2rr + bb holds pair 4o + 2bb + rr. Loads for
                # rr=0 (needed by the qp=0 quads) are issued first so
                # compute can start after half the batch has landed.
                if probe_no_dma:
                    pass
                elif probe_contig_loads:
                    # timing probe only: plain contiguous cast-loads
                    nc.gpsimd.dma_start(
                        out=in_r,
                        in_=x[b, : C2 // 2].rearrange(
                            "(pp c) h w -> pp (c h w)", pp=128
                        ),
                    )
                    nc.gpsimd.dma_start(
                        out=in_i,
                        in_=x[b, C2 // 2 :].rearrange(
                            "(pp c) h w -> pp (c h w)", pp=128
                        ),
                    )
                for rr in (() if (probe_contig_loads or probe_no_dma) else range(2)):
                    for tile, lohalf in ((in_r, x[b, : C2 // 2]),
                                         (in_i, x[b, C2 // 2 :])):
                        src = lohalf.rearrange(
                            "(o bb rr two) h w -> o bb rr two h w",
                            o=16, bb=2, rr=2, two=2,
                        )
                        if load_mode == "l2":
                            # slot 32rr + 2o + bb: one contiguous-dst DMA/rr
                            nc.gpsimd.dma_start(
                                out=tile[:, 2048 * rr : 2048 * rr + 2048],
                                # (o,bb,two,h,w) -> (two,h,o,bb,w)
                                in_=src[:, :, rr].transpose([2, 3, 0, 1, 4]),
                                single_packet=sp_loads,
                            )
                            continue
                        dst = tile.rearrange(
                            "p (o sw) -> p o sw", o=16, sw=256
                        )
                        for bb in range(2):
                            so = 64 * (2 * rr + bb)
                            nc.gpsimd.dma_start(
                                out=dst[:, :, so : so + 64],
                                # (o,two,h,w) -> (two,h,o,w)
                                in_=src[:, bb, rr].transpose([1, 2, 0, 3]),
                                single_packet=sp_loads,
                            )

                sb_out = opool.tile([128, 2048], F32, tag="sb_out")
                if probe_no_compute:
                    nc.vector.memset(sb_out, 0.0)
                if probe_no_dma:
                    nc.gpsimd.memset(in_r[:, :], 0.0)
                    nc.gpsimd.memset(in_i[:, :], 0.0)
                for q in (() if probe_no_compute else range(C2 // 8)):
                    # quad (o, qp): complex channels {8o+2qp, +1, +4, +5}
                    # qp-major order: all rr=0-fed quads first
                    if interleave_qp:
                        qp, o = q % 2, q // 2
                    else:
                        qp, o = q // 16, q % 16
                    if load_mode == "l2":
                        lo = 2048 * qp + 128 * o
                    else:
                        lo = 256 * o + 128 * qp
                    psum1 = p1pool.tile([128, 128], F32, tag="ps1")
                    nc.tensor.matmul(
                        out=psum1,
                        lhsT=in_r[:, lo : lo + 128],
                        rhs=d1rb,
                        start=True,
                        stop=False,
                        tile_position=None if no_tilepos else (0, 0),
                    )
                    nc.tensor.matmul(
                        out=psum1,
                        lhsT=in_i[:, lo : lo + 128],
                        rhs=d1ib,
                        start=False,
                        stop=True,
                        tile_position=None if no_tilepos else (0, 0),
                    )
                    sb1 = s1pool.tile([128, 128], BF16, tag="sb1")
                    nc.vector.tensor_copy(out=sb1, in_=psum1)

                    psum2 = p2pool.tile([128, 128], F32, tag="ps2")
                    cb = 64 * qp
                    nc.tensor.matmul(
                        out=psum2[cb : cb + 64, :],
                        lhsT=sb1[:, 0:64],
                        rhs=d2rb,
                        start=True,
                        stop=False,
                        tile_position=None if no_tilepos else (0, cb),
                    )
                    nc.tensor.matmul(
                        out=psum2[cb : cb + 64, :],
                        lhsT=sb1[:, 64:128],
                        rhs=d2ib,
                        start=False,
                        stop=True,
                        tile_position=None if no_tilepos else (0, cb),
                    )
                    nc.scalar.copy(
                        out=sb_out[cb : cb + 64, 128 * o : 128 * (o + 1)],
                        in_=psum2[cb : cb + 64, :],
                    )
                # channel = 128*ri + 8*o + 4*t + 2*qp + s ; partitions (qp s h)
                sbv = sb_out.rearrange(
                    "p (o t ri w) -> p (o t) ri w", o=16, t=2, ri=2, w=WP
                )
                st_eng = nc.gpsimd if gp_stores else nc.sync
                if probe_no_dma:
                    continue
                if probe_contig_stores:
                    st_eng.dma_start(
                        out=y[b].rearrange("(pp c) h w -> pp (c h w)", pp=128),
                        in_=sb_out,
                    )
                for ri in (() if probe_contig_stores else range(2)):
                    st_eng.dma_start(
                        out=y[b, 128 * ri : 128 * (ri + 1)].rearrange(
                            "(o t qp s) h w -> (qp s h) (o t) w",
                            o=16, t=2, qp=2, s=2,
                        ),
                        in_=sbv[:, :, ri, :],
                        single_packet=sp_stores,
                    )
    if split_waits:
        _split_multi_waits(nc)
    return nc


_CACHED = {}


def _get_program():
    if "nc" not in _CACHED:
        _CACHED["nc"] = build_program()
        _CACHED["consts"] = _dft_constants()
    return _CACHED["nc"], _CACHED["consts"]


def kernel(x: np.ndarray) -> np.ndarray:
    assert x.shape == (B_FULL, C2, H, W) and x.dtype == np.float32
    nc, dmats = _get_program()
    x = np.ascontiguousarray(x)
    in_maps = [
        {"x": x[BPC * k : BPC * (k + 1)], "dmats": dmats}
        for k in range(N_CORES)
    ]
    res = run_bass_kernel_spmd(nc, in_maps, list(range(N_CORES)))
    out = np.concatenate(
        [res.results[k]["y"] for k in range(N_CORES)], axis=0
    )
    return out.astype(np.float32, copy=False)


if __name__ == "__main__":
    rng = np.random.default_rng(0)
    x = rng.standard_normal((B_FULL, C2, H, W)).astype(np.float32)
    y = kernel(x)
    print("kernel output", y.shape, y.dtype)



# revision 13
# speedup vs baseline: 1.0893x; 1.0893x over previous
"""Spectral pooling (FFT2 -> crop low freqs -> IFFT2) as dense DFT matmuls on TRN2.

Input  x: (32, 256, 64, 64) fp32  -- channels 0:128 real part, 128:256 imag part
Output y: (32, 256, 32, 32) fp32

Math: per complex image X (64x64), Y = A @ X @ A.T with
  A = sqrt(1/(64*32)) * IDFT32 @ Crop @ DFT64   (32x64 complex)
Sharding: batch dim across 8 cores (4 batches/core), no communication.

Layout scheme (all DMA access patterns <= 3 dims, HBM-side chunks >= 512B):
  Loads put h-row PAIRS (or QUADS for the first NQ channel-blocks) of each
  channel in one partition line: partition = (par, h//2), free =
  (pg, sl, gab, h%2, w); the channel map c = pg*16 + sl*8 + gab*4 + par makes
  both the load and store APs merge into 3 dims with 512B last-dim chunks,
  and the fp32->bf16 cast rides the SWDGE descriptors. A small on-chip
  permute copy swaps (sl, h%2) so stage-1 matmuls get a contiguous
  128-column stationary operand (the ISA allows only one free dim there).
  Stage 1 (contract h) does 4 accumulating matmuls (Xr/Xi x h-parity) per
  8-channel group; stage 2 (contract w) computes Yr into PSUM partitions
  0:64 and Yi into 64:128 (complex combine via +/- DFT constant halves and
  tile_position), so the single PSUM->SBUF copy per group and the stores
  stay partition-aligned.
"""

import math

import numpy as np

from concourse import bass, mybir
from concourse.bass_utils import run_bass_kernel_spmd
from concourse.tile import TileContext

N_CORES = 8
B_FULL, C2, H, W = 32, 256, 64, 64
HP, WP = 32, 32
BPC = B_FULL // N_CORES  # batches per core

NQ = 0  # channel-blocks (of 16) per batch loaded with h-quad layout

F32 = mybir.dt.float32
BF16 = mybir.dt.bfloat16

# const column layout in dmats [128, 5376]
_D1P_O = 0      # 4 x 256  (kind*2+s), cols (ri2, p4, par4, f1q8)
_D1Q_O = 1024   # 8 x 512  (kind*4+shr), cols (ri2, p4, par8, f1q8)
_D2_O = 5120    # 4 x 64   (half*2+ri), cols (sl2, f2_32)
_NCOL = 5376


def _split_multi_waits(nc):
    """This walrus build rejects instructions carrying more than one semaphore
    wait. Hoist extra waits onto same-engine NOPs inserted just before the
    instruction (engine queues execute in order, so blocking is equivalent)."""
    n_split = 0
    for f in nc.m.functions:
        for bb in f.blocks:
            insts = bb.instructions
            out = []
            for inst in insts:
                si = inst.sync_info
                waits = list(si.on_wait) if si and si.on_wait else []
                if len(waits) > 1:
                    si.on_wait = waits[-1:]
                    for w in waits[:-1]:
                        nop = mybir.InstNoOp(
                            name=nc.get_next_instruction_name(),
                            ins=[],
                            outs=[],
                            engine=inst.engine,
                            sync_info=mybir.SyncInfo(on_wait=[w], on_update=[]),
                        )
                        out.append(nop)
                        n_split += 1
                out.append(inst)
            if len(out) != len(insts):
                insts[:] = out
    return n_split


def _dft_constants():
    """bf16 [128, 5376]: stage-1 pair/quad and stage-2 half constants."""
    topf = int(math.ceil(H * 0.5 / 2))  # 16
    midf = H // 2 + topf  # 48
    F = np.exp(-2j * np.pi * np.outer(np.arange(H), np.arange(H)) / H)
    G = np.exp(2j * np.pi * np.outer(np.arange(HP), np.arange(HP)) / HP)
    keep = list(range(topf)) + list(range(midf, H))
    S = np.zeros((HP, H))
    S[np.arange(HP), keep] = 1
    A = (G @ S @ F) / np.sqrt(H * W * HP * WP) ** 0.5
    Ar = A.real.astype(np.float32)  # [32, 64]
    Ai = A.imag.astype(np.float32)

    # f1 reindex: col-position (p, f1q) holds frequency f1 = 4*f1q + p
    perm = np.empty(32, np.int64)  # perm[p*8 + f1q] = 4*f1q + p
    for p in range(4):
        for f1q in range(8):
            perm[p * 8 + f1q] = 4 * f1q + p
    Arp, Aip = Ar[perm], Ai[perm]

    dm = np.zeros((128, _NCOL), np.float32)
    # D1 pair [(par4,he32), (ri2, p4, par4, f1q8)], value at h = 2*he + s
    for kind in range(2):
        for s in range(2):
            o = _D1P_O + (kind * 2 + s) * 256
            for par in range(4):
                rows = slice(par * 32, par * 32 + 32)  # he
                h = 2 * np.arange(32) + s
                v0, v1 = (Arp, Aip) if kind == 0 else (-Aip, Arp)
                # cols ri*128 + (p*8+f1q=j)*4?? -> col = ri*128 + j4*32??
                for j in range(32):  # j = p*8 + f1q -> col ri*128 + p*32 + par*8 + f1q
                    p, f1q = j // 8, j % 8
                    dm[rows, o + p * 32 + par * 8 + f1q] = v0[j][h]
                    dm[rows, o + 128 + p * 32 + par * 8 + f1q] = v1[j][h]
    # D1 quad [(par8,hq16), (ri2, p4, par8, f1q8)], value at h = 4*hq + shr
    for kind in range(2):
        for shr in range(4):
            o = _D1Q_O + (kind * 4 + shr) * 512
            for par in range(8):
                rows = slice(par * 16, par * 16 + 16)  # hq
                h = 4 * np.arange(16) + shr
                v0, v1 = (Arp, Aip) if kind == 0 else (-Aip, Arp)
                for j in range(32):
                    p, f1q = j // 8, j % 8
                    dm[rows, o + p * 64 + par * 8 + f1q] = v0[j][h]
                    dm[rows, o + 256 + p * 64 + par * 8 + f1q] = v1[j][h]
    # D2 [(sl2,w64), (sl2,f2_32)] x (half, ri)
    for half in range(2):
        for ri in range(2):
            o = _D2_O + (half * 2 + ri) * 64
            v = (Ar, -Ai)[ri] if half == 0 else (Ai, Ar)[ri]
            for sl in range(2):
                dm[sl * 64 : sl * 64 + 64, o + sl * 32 : o + sl * 32 + 32] = v.T
    return dm.astype(mybir.dt.np(BF16))


def build_program(reps: int = 1, nq: int = NQ, split_waits: bool = True,
                  probe_no_compute: bool = False,
                  probe_no_dma: bool = False):
    """reps > 1 unrolls the whole pipeline in-NEFF over the same data so the
    marginal cost per rep can be measured without the ~65ms axon dispatch
    overhead."""
    npair = 8 - nq
    nc = bass.Bass("TRN2", target_bir_lowering=False, debug=False)
    x = nc.dram_tensor("x", [BPC, C2, H, W], F32, kind="ExternalInput").ap()
    dm = nc.dram_tensor("dmats", [128, _NCOL], BF16, kind="ExternalInput").ap()
    y = nc.dram_tensor("y", [BPC, C2, HP, WP], F32, kind="ExternalOutput").ap()

    with TileContext(nc) as tc:
        with (
            tc.tile_pool(name="consts", bufs=1) as cpool,
            tc.tile_pool(name="inp", bufs=2) as ipool,
            tc.tile_pool(name="sb1", bufs=4) as s1pool,
            tc.tile_pool(name="sbout", bufs=2) as opool,
            tc.tile_pool(name="ps1", bufs=2, space="PSUM") as p1pool,
            tc.tile_pool(name="ps2", bufs=4, space="PSUM") as p2pool,
        ):
            dmb = cpool.tile([128, _NCOL], BF16, tag="dmb")
            nc.sync.dma_start(out=dmb, in_=dm)
            d1p = [dmb[:, _D1P_O + j * 256 : _D1P_O + (j + 1) * 256]
                   for j in range(4)]
            d1q = [dmb[:, _D1Q_O + j * 512 : _D1Q_O + (j + 1) * 512]
                   for j in range(8)]
            d2 = [dmb[:, _D2_O + j * 64 : _D2_O + (j + 1) * 64]
                  for j in range(4)]

            def in_tiles(pfx):
                t = {}
                if npair:
                    t["pr"] = ipool.tile([128, npair * 512], BF16,
                                         tag=pfx + "pr", name=pfx + "pr")
                    t["pi"] = ipool.tile([128, npair * 512], BF16,
                                         tag=pfx + "pi", name=pfx + "pi")
                if nq:
                    t["qr"] = ipool.tile([128, nq * 512], BF16,
                                         tag=pfx + "qr", name=pfx + "qr")
                    t["qi"] = ipool.tile([128, nq * 512], BF16,
                                         tag=pfx + "qi", name=pfx + "qi")
                return t

            cp_cnt = [0]

            def cp_eng():
                cp_cnt[0] += 1
                return nc.vector if cp_cnt[0] % 2 else nc.scalar

            def copy(eng, out, in_):
                if eng is nc.scalar:
                    eng.copy(out=out, in_=in_)
                else:
                    eng.tensor_copy(out=out, in_=in_)

            if probe_no_dma:
                raw_fixed = in_tiles("r")
                for t in raw_fixed.values():
                    nc.vector.memset(t, 0.0)
                sbo_fixed = opool.tile([128, 2048], F32, tag="sbo", name="sbo")

            for b in [b for _ in range(reps) for b in range(BPC)]:
                if probe_no_dma:
                    raw = raw_fixed
                else:
                    raw = in_tiles("r")
                    for kind, key in ((0, "pr"), (1, "pi")):
                        half = x[b, 128 * kind : 128 * kind + 128]
                        if npair:
                            nc.gpsimd.dma_start(
                                out=raw[key],
                                # channel c = pg*16 + sl*8 + gab*4 + par:
                                # (pg sl gab par he s w) -> (par he pg sl gab s w)
                                # merges to [(par he)=128, (pg sl gab)=3npair, (s w)]
                                in_=half[nq * 16 :].rearrange(
                                    "(pg sl gab par) (he s) w"
                                    " -> pg sl gab par he s w",
                                    pg=npair, sl=2, gab=2, par=4, he=32, s=2,
                                ).transpose([3, 4, 0, 1, 2, 5, 6]),
                            )
                        if nq:
                            nc.gpsimd.dma_start(
                                out=raw["qr" if kind == 0 else "qi"],
                                # channel c = pg*16 + sl*8 + par:
                                # (pg sl par hq s w) -> (par hq pg sl s w)
                                in_=half[: nq * 16].rearrange(
                                    "(pg sl par) (hq s) w"
                                    " -> pg sl par hq s w",
                                    pg=nq, sl=2, par=8, hq=16, s=4,
                                ).transpose([2, 3, 0, 1, 4, 5]),
                            )

                if probe_no_dma:
                    sbo = sbo_fixed
                else:
                    sbo = opool.tile([128, 2048], F32, tag="sbo", name="sbo")
                if probe_no_compute:
                    nc.vector.memset(sbo, 0.0)
                else:
                    # permute (sl, s) -> (s, sl) so stage-1 lhsT is contiguous
                    perm = in_tiles("m")
                    for kind, rk, mk in ((0, "pr", "pr"), (1, "pi", "pi")):
                        if npair:
                            rv = raw[rk].rearrange(
                                "k (pg sl gab s w) -> k pg sl gab s w",
                                pg=npair, sl=2, gab=2, s=2, w=64)
                            for pg in range(npair):
                                for gab in range(2):
                                    copy(cp_eng(),
                                         out=perm[mk][:, pg * 512 + gab * 256 :
                                                      pg * 512 + gab * 256 + 256],
                                         in_=rv[:, pg, :, gab, :, :]
                                         .transpose([0, 2, 1, 3]))
                        if nq:
                            qk = "qr" if kind == 0 else "qi"
                            rv = raw[qk].rearrange(
                                "k (pg sl s w) -> k pg sl s w",
                                pg=nq, sl=2, s=4, w=64)
                            for pg in range(nq):
                                copy(cp_eng(),
                                     out=perm[qk][:, pg * 512 : pg * 512 + 512],
                                     in_=rv[:, pg, :, :, :]
                                     .transpose([0, 2, 1, 3]))

                    for pg in range(8):
                        sb1 = s1pool.tile([128, 512], BF16, tag="sb1",
                                          name="sb1")
                        if pg < nq:  # h-quad block: perm layout (pg, s4, sl, w)
                            ps1 = p1pool.tile([128, 512], F32, tag="ps1q",
                                              name="ps1q")
                            for j, (kind, shr) in enumerate(
                                    (k, s) for k in range(2) for s in range(4)):
                                t = perm["qr" if kind == 0 else "qi"]
                                o = pg * 512 + shr * 128
                                nc.tensor.matmul(
                                    out=ps1,
                                    lhsT=t[:, o : o + 128],
                                    rhs=d1q[kind * 4 + shr],
                                    start=(j == 0),
                                    stop=(j == 7),
                                )
                            copy(cp_eng(), out=sb1, in_=ps1)
                        else:  # h-pair block: perm layout (pg, gab, s, sl, w)
                            pgp = pg - nq
                            for gab in range(2):
                                ps1 = p1pool.tile([128, 256], F32, tag="ps1p",
                                                  name="ps1p")
                                for j, (kind, s) in enumerate(
                                        (k, s) for k in range(2) for s in range(2)):
                                    t = perm["pr" if kind == 0 else "pi"]
                                    o = pgp * 512 + gab * 256 + s * 128
                                    nc.tensor.matmul(
                                        out=ps1,
                                        lhsT=t[:, o : o + 128],
                                        rhs=d1p[kind * 2 + s],
                                        start=(j == 0),
                                        stop=(j == 3),
                                    )
                                # scatter cols (ri, p, pf32) -> sb1 (ri, p, gab, pf32)
                                copy(cp_eng(),
                                     out=sb1.rearrange(
                                         "k (ri p gab pf) -> k ri p gab pf",
                                         ri=2, p=4, gab=2, pf=32)[:, :, :, gab, :],
                                     in_=ps1.rearrange(
                                         "k (ri p pf) -> k ri p pf",
                                         ri=2, p=4, pf=32))

                        ps2 = p2pool.tile([128, 256], F32, tag="ps2",
                                          name="ps2")
                        for p in range(4):
                            for hf in range(2):
                                for ri in range(2):
                                    nc.tensor.matmul(
                                        out=ps2[64 * hf : 64 * hf + 64,
                                                64 * p : 64 * p + 64],
                                        lhsT=sb1[:, ri * 256 + p * 64 :
                                                 ri * 256 + p * 64 + 64],
                                        rhs=d2[hf * 2 + ri],
                                        start=(ri == 0),
                                        stop=(ri == 1),
                                        tile_position=(0, 64 * hf),
                                    )
                        copy(cp_eng(),
                             out=sbo.rearrange(
                                 "m (pg sl p f2) -> m pg sl p f2",
                                 pg=8, sl=2, p=4, f2=32)[:, pg],
                             in_=ps2.rearrange(
                                 "m (p sl f2) -> m sl p f2",
                                 p=4, sl=2, f2=32))

                if not probe_no_dma:
                    for half in range(2):
                        nc.sync.dma_start(
                            # channel c = pg*16 + sl*8 + gp:
                            # (pg sl gp f1q p f2) -> (gp f1q pg sl p f2)
                            # merges to [(gp f1q)=64, (pg sl)=16, (p f2)=128]
                            out=y[b, 128 * half : 128 * half + 128].rearrange(
                                "(pg sl gp) (f1q p) f2 -> pg sl gp f1q p f2",
                                pg=8, sl=2, gp=8, f1q=8, p=4,
                            ).transpose([2, 3, 0, 1, 4, 5]),
                            in_=sbo[64 * half : 64 * half + 64, :],
                        )
    if split_waits:
        _split_multi_waits(nc)
    return nc


_CACHED = {}


def _get_program():
    if "nc" not in _CACHED:
        _CACHED["nc"] = build_program()
        _CACHED["consts"] = _dft_constants()
    return _CACHED["nc"], _CACHED["consts"]


def kernel(x: np.ndarray) -> np.ndarray:
    assert x.shape == (B_FULL, C2, H, W) and x.dtype == np.float32
    nc, dmats = _get_program()
    x = np.ascontiguousarray(x)
    in_maps = [
        {"x": x[BPC * k : BPC * (k + 1)], "dmats": dmats}
        for k in range(N_CORES)
    ]
    res = run_bass_kernel_spmd(nc, in_maps, list(range(N_CORES)))
    out = np.concatenate(
        [res.results[k]["y"] for k in range(N_CORES)], axis=0
    )
    return out.astype(np.float32, copy=False)


if __name__ == "__main__":
    rng = np.random.default_rng(0)
    x = rng.standard_normal((B_FULL, C2, H, W)).astype(np.float32)
    y = kernel(x)
    print("kernel output", y.shape, y.dtype)


# revision 27
# speedup vs baseline: 1.1842x; 1.0871x over previous
"""Spectral pooling (FFT2 -> crop low freqs -> IFFT2) as dense DFT matmuls on TRN2.

Input  x: (32, 256, 64, 64) fp32  -- channels 0:128 real part, 128:256 imag part
Output y: (32, 256, 32, 32) fp32

Math: per complex image X (64x64), Y = A @ X @ A.T with
  A = sqrt(1/(64*32)) * IDFT32 @ Crop @ DFT64   (32x64 complex)
Sharding: batch dim across 8 cores (4 batches/core), no communication.

Layout scheme (all DMA access patterns <= 3 dims, HBM-side chunks >= 512B):
  Loads put h-row PAIRS (or QUADS for the first NQ channel-blocks) of each
  channel in one partition line: partition = (par, h//2), free =
  (pg, sl, gab, h%2, w); the channel map c = pg*16 + sl*8 + gab*4 + par makes
  both the load and store APs merge into 3 dims with 512B last-dim chunks,
  and the fp32->bf16 cast rides the SWDGE descriptors. A small on-chip
  permute copy swaps (sl, h%2) so stage-1 matmuls get a contiguous
  128-column stationary operand (the ISA allows only one free dim there).
  Stage 1 (contract h) does 4 accumulating matmuls (Xr/Xi x h-parity) per
  8-channel group; stage 2 (contract w) computes Yr into PSUM partitions
  0:64 and Yi into 64:128 (complex combine via +/- DFT constant halves and
  tile_position), so the single PSUM->SBUF copy per group and the stores
  stay partition-aligned.
"""

import math

import numpy as np

from concourse import bass, mybir
from concourse.bass_utils import run_bass_kernel_spmd
from concourse.tile import TileContext

N_CORES = 8
B_FULL, C2, H, W = 32, 256, 64, 64
HP, WP = 32, 32
BPC = B_FULL // N_CORES  # batches per core

NQ = 2  # channel-blocks (of 16) per batch loaded with h-quad layout

F32 = mybir.dt.float32
BF16 = mybir.dt.bfloat16

# const column layout in dmats [128, 5376]
_D1P_O = 0      # 4 x 256  (kind*2+s), cols (ri2, p4, par4, f1q8)
_D1Q_O = 1024   # 8 x 512  (kind*4+shr), cols (ri2, p4, par8, f1q8)
_D2_O = 5120    # 4 x 64   (half*2+ri), cols (sl2, f2_32)
_NCOL = 5376


def _split_multi_waits(nc):
    """This walrus build rejects instructions carrying more than one semaphore
    wait. Hoist extra waits onto same-engine NOPs inserted just before the
    instruction (engine queues execute in order, so blocking is equivalent)."""
    n_split = 0
    for f in nc.m.functions:
        for bb in f.blocks:
            insts = bb.instructions
            out = []
            for inst in insts:
                si = inst.sync_info
                waits = list(si.on_wait) if si and si.on_wait else []
                if len(waits) > 1:
                    si.on_wait = waits[-1:]
                    for w in waits[:-1]:
                        nop = mybir.InstNoOp(
                            name=nc.get_next_instruction_name(),
                            ins=[],
                            outs=[],
                            engine=inst.engine,
                            sync_info=mybir.SyncInfo(on_wait=[w], on_update=[]),
                        )
                        out.append(nop)
                        n_split += 1
                out.append(inst)
            if len(out) != len(insts):
                insts[:] = out
    return n_split


def _dft_constants():
    """bf16 [128, 5376]: stage-1 pair/quad and stage-2 half constants."""
    topf = int(math.ceil(H * 0.5 / 2))  # 16
    midf = H // 2 + topf  # 48
    F = np.exp(-2j * np.pi * np.outer(np.arange(H), np.arange(H)) / H)
    G = np.exp(2j * np.pi * np.outer(np.arange(HP), np.arange(HP)) / HP)
    keep = list(range(topf)) + list(range(midf, H))
    S = np.zeros((HP, H))
    S[np.arange(HP), keep] = 1
    A = (G @ S @ F) / np.sqrt(H * W * HP * WP) ** 0.5
    Ar = A.real.astype(np.float32)  # [32, 64]
    Ai = A.imag.astype(np.float32)

    # f1 reindex: col-position (p, f1q) holds frequency f1 = 4*f1q + p
    perm = np.empty(32, np.int64)  # perm[p*8 + f1q] = 4*f1q + p
    for p in range(4):
        for f1q in range(8):
            perm[p * 8 + f1q] = 4 * f1q + p
    Arp, Aip = Ar[perm], Ai[perm]

    dm = np.zeros((128, _NCOL), np.float32)
    # D1 pair [(par4,he32), (ri2, p4, par4, f1q8)], value at h = 2*he + s
    for kind in range(2):
        for s in range(2):
            o = _D1P_O + (kind * 2 + s) * 256
            for par in range(4):
                rows = slice(par * 32, par * 32 + 32)  # he
                h = 2 * np.arange(32) + s
                v0, v1 = (Arp, Aip) if kind == 0 else (-Aip, Arp)
                # cols ri*128 + (p*8+f1q=j)*4?? -> col = ri*128 + j4*32??
                for j in range(32):  # j = p*8 + f1q -> col ri*128 + p*32 + par*8 + f1q
                    p, f1q = j // 8, j % 8
                    dm[rows, o + p * 32 + par * 8 + f1q] = v0[j][h]
                    dm[rows, o + 128 + p * 32 + par * 8 + f1q] = v1[j][h]
    # D1 quad [(par8,hq16), (ri2, p4, par8, f1q8)], value at h = 4*hq + shr
    for kind in range(2):
        for shr in range(4):
            o = _D1Q_O + (kind * 4 + shr) * 512
            for par in range(8):
                rows = slice(par * 16, par * 16 + 16)  # hq
                h = 4 * np.arange(16) + shr
                v0, v1 = (Arp, Aip) if kind == 0 else (-Aip, Arp)
                for j in range(32):
                    p, f1q = j // 8, j % 8
                    dm[rows, o + p * 64 + par * 8 + f1q] = v0[j][h]
                    dm[rows, o + 256 + p * 64 + par * 8 + f1q] = v1[j][h]
    # D2 per ri: [(sl2,w64), (hf2, sl2, f2_32)=128]
    for ri in range(2):
        for half in range(2):
            o = _D2_O + ri * 128 + half * 64
            v = (Ar, -Ai)[ri] if half == 0 else (Ai, Ar)[ri]
            for sl in range(2):
                dm[sl * 64 : sl * 64 + 64, o + sl * 32 : o + sl * 32 + 32] = v.T
    return dm.astype(mybir.dt.np(BF16))


def build_program(reps: int = 1, nq: int = NQ, split_waits: bool = True,
                  probe_no_compute: bool = False,
                  probe_no_dma: bool = False,
                  ib: int = 2, s1b: int = 8, p1b: int = 4, p2b: int = 4,
                  ob: int = 3, s2_mode: str = "tp16",
                  perm_eng: str = "v", s1_eng: str = "alt", s2_eng: str = "a",
                  big_perm: bool = False):
    """reps > 1 unrolls the whole pipeline in-NEFF over the same data so the
    marginal cost per rep can be measured without the ~65ms axon dispatch
    overhead."""
    npair = 8 - nq
    nc = bass.Bass("TRN2", target_bir_lowering=False, debug=False)
    x = nc.dram_tensor("x", [BPC, C2, H, W], F32, kind="ExternalInput").ap()
    dm = nc.dram_tensor("dmats", [128, _NCOL], BF16, kind="ExternalInput").ap()
    y = nc.dram_tensor("y", [BPC, C2, HP, WP], F32, kind="ExternalOutput").ap()

    with TileContext(nc) as tc:
        with (
            tc.tile_pool(name="consts", bufs=1) as cpool,
            tc.tile_pool(name="inp", bufs=ib) as ipool,
            tc.tile_pool(name="sb1", bufs=s1b) as s1pool,
            tc.tile_pool(name="sbout", bufs=ob) as opool,
            tc.tile_pool(name="ps1", bufs=p1b, space="PSUM") as p1pool,
            tc.tile_pool(name="ps2", bufs=p2b, space="PSUM") as p2pool,
        ):
            dmb = cpool.tile([128, _NCOL], BF16, tag="dmb")
            nc.sync.dma_start(out=dmb, in_=dm)
            d1p = [dmb[:, _D1P_O + j * 256 : _D1P_O + (j + 1) * 256]
                   for j in range(4)]
            d1q = [dmb[:, _D1Q_O + j * 512 : _D1Q_O + (j + 1) * 512]
                   for j in range(8)]
            d2 = [dmb[:, _D2_O + j * 128 : _D2_O + (j + 1) * 128]
                  for j in range(2)]

            def in_tiles(pfx):
                t = {}
                if npair:
                    t["pr"] = ipool.tile([128, npair * 512], BF16,
                                         tag=pfx + "pr", name=pfx + "pr")
                    t["pi"] = ipool.tile([128, npair * 512], BF16,
                                         tag=pfx + "pi", name=pfx + "pi")
                if nq:
                    t["qr"] = ipool.tile([128, nq * 512], BF16,
                                         tag=pfx + "qr", name=pfx + "qr")
                    t["qi"] = ipool.tile([128, nq * 512], BF16,
                                         tag=pfx + "qi", name=pfx + "qi")
                return t

            cp_cnt = [0]

            def cp_eng(which):
                if which == "alt":
                    cp_cnt[0] += 1
                    which = "va"[cp_cnt[0] % 2]
                return {"v": nc.vector, "a": nc.scalar, "p": nc.gpsimd}[which]

            def copy(eng, out, in_):
                if eng is nc.scalar:
                    eng.copy(out=out, in_=in_)
                else:
                    eng.tensor_copy(out=out, in_=in_)

            if probe_no_dma:
                raw_fixed = in_tiles("r")
                for t in raw_fixed.values():
                    nc.vector.memset(t, 0.0)
                sbo_fixed = opool.tile([128, 2048], F32, tag="sbo", name="sbo")

            for b in [b for _ in range(reps) for b in range(BPC)]:
                if probe_no_dma:
                    raw = raw_fixed
                else:
                    raw = in_tiles("r")
                    for kind, key in ((0, "pr"), (1, "pi")):
                        half = x[b, 128 * kind : 128 * kind + 128]
                        if npair:
                            nc.gpsimd.dma_start(
                                out=raw[key],
                                # channel c = pg*16 + sl*8 + gab*4 + par:
                                # (pg sl gab par he s w) -> (par he pg sl gab s w)
                                # merges to [(par he)=128, (pg sl gab)=3npair, (s w)]
                                in_=half[nq * 16 :].rearrange(
                                    "(pg sl gab par) (he s) w"
                                    " -> pg sl gab par he s w",
                                    pg=npair, sl=2, gab=2, par=4, he=32, s=2,
                                ).transpose([3, 4, 0, 1, 2, 5, 6]),
                            )
                        if nq:
                            nc.gpsimd.dma_start(
                                out=raw["qr" if kind == 0 else "qi"],
                                # channel c = pg*16 + sl*8 + par:
                                # (pg sl par hq s w) -> (par hq pg sl s w)
                                in_=half[: nq * 16].rearrange(
                                    "(pg sl par) (hq s) w"
                                    " -> pg sl par hq s w",
                                    pg=nq, sl=2, par=8, hq=16, s=4,
                                ).transpose([2, 3, 0, 1, 4, 5]),
                            )

                if probe_no_dma:
                    sbo = sbo_fixed
                else:
                    sbo = opool.tile([128, 2048], F32, tag="sbo", name="sbo")
                if probe_no_compute:
                    nc.vector.memset(sbo, 0.0)
                else:
                    # permute (sl, s) -> (s, sl) so stage-1 lhsT is contiguous
                    perm = in_tiles("m")
                    for kind, rk, mk in ((0, "pr", "pr"), (1, "pi", "pi")):
                        if npair:
                            rv = raw[rk].rearrange(
                                "k (pg sl gab s w) -> k pg sl gab s w",
                                pg=npair, sl=2, gab=2, s=2, w=64)
                            for pg in range(npair):
                                if big_perm:
                                    # one [128, 512] copy (4-dim src AP)
                                    copy(cp_eng(perm_eng),
                                         out=perm[mk][:, pg * 512 :
                                                      pg * 512 + 512],
                                         in_=rv[:, pg]
                                         .transpose([0, 2, 3, 1, 4]))
                                    continue
                                for gab in range(2):
                                    copy(cp_eng(perm_eng),
                                         out=perm[mk][:, pg * 512 + gab * 256 :
                                                      pg * 512 + gab * 256 + 256],
                                         in_=rv[:, pg, :, gab, :, :]
                                         .transpose([0, 2, 1, 3]))
                        if nq:
                            qk = "qr" if kind == 0 else "qi"
                            rv = raw[qk].rearrange(
                                "k (pg sl s w) -> k pg sl s w",
                                pg=nq, sl=2, s=4, w=64)
                            for pg in range(nq):
                                copy(cp_eng(perm_eng),
                                     out=perm[qk][:, pg * 512 : pg * 512 + 512],
                                     in_=rv[:, pg, :, :, :]
                                     .transpose([0, 2, 1, 3]))

                    for pg in range(8):
                        sb1 = s1pool.tile([128, 512], BF16, tag="sb1",
                                          name="sb1")
                        if pg < nq:  # h-quad block: perm layout (pg, s4, sl, w)
                            ps1 = p1pool.tile([128, 512], F32, tag="ps1",
                                              name="ps1q")
                            for j, (kind, shr) in enumerate(
                                    (k, s) for k in range(2) for s in range(4)):
                                t = perm["qr" if kind == 0 else "qi"]
                                o = pg * 512 + shr * 128
                                nc.tensor.matmul(
                                    out=ps1,
                                    lhsT=t[:, o : o + 128],
                                    rhs=d1q[kind * 4 + shr],
                                    start=(j == 0),
                                    stop=(j == 7),
                                )
                            copy(cp_eng(s1_eng), out=sb1, in_=ps1)
                        else:  # h-pair block: perm layout (pg, gab, s, sl, w)
                            pgp = pg - nq
                            for gab in range(2):
                                ps1f = p1pool.tile([128, 512], F32, tag="ps1",
                                                   name="ps1p")
                                ps1 = ps1f[:, 0:256]
                                for j, (kind, s) in enumerate(
                                        (k, s) for k in range(2) for s in range(2)):
                                    t = perm["pr" if kind == 0 else "pi"]
                                    o = pgp * 512 + gab * 256 + s * 128
                                    nc.tensor.matmul(
                                        out=ps1,
                                        lhsT=t[:, o : o + 128],
                                        rhs=d1p[kind * 2 + s],
                                        start=(j == 0),
                                        stop=(j == 3),
                                    )
                                # scatter cols (ri, p, pf32) -> sb1 (ri, p, gab, pf32)
                                copy(cp_eng(s1_eng),
                                     out=sb1.rearrange(
                                         "k (ri p gab pf) -> k ri p gab pf",
                                         ri=2, p=4, gab=2, pf=32)[:, :, :, gab, :],
                                     in_=ps1.rearrange(
                                         "k (ri p pf) -> k ri p pf",
                                         ri=2, p=4, pf=32))

                        sbov = sbo.rearrange(
                            "m (pg sl p f2) -> m pg sl p f2",
                            pg=8, sl=2, p=4, f2=32)
                        if s2_mode == "xb8":
                            ps2f = p2pool.tile([128, 512], F32, tag="ps2",
                                               name="ps2")
                            ps2 = ps2f[0:64, :]
                            for p in range(4):
                                for ri in range(2):
                                    nc.tensor.matmul(
                                        out=ps2[:, 128 * p : 128 * p + 128],
                                        lhsT=sb1[:, ri * 256 + p * 64 :
                                                 ri * 256 + p * 64 + 64],
                                        rhs=d2[ri],
                                        start=(ri == 0),
                                        stop=(ri == 1),
                                    )
                            # ps2 cols (p4, hf2, sl2, f2_32); Yi -> sbo[64:128]
                            # via cross-partition-base copy (verified legal)
                            ps2v = ps2.rearrange(
                                "m (p hf sl f2) -> m hf sl p f2",
                                p=4, hf=2, sl=2, f2=32)
                            for hf in range(2):
                                copy(cp_eng(s2_eng),
                                     out=sbov[64 * hf : 64 * hf + 64, pg],
                                     in_=ps2v[:, hf])
                        else:  # tp16: tile_position halves, single copy
                            ps2f = p2pool.tile([128, 256], F32, tag="ps2",
                                               name="ps2")
                            for p in range(4):
                                for hf in range(2):
                                    for ri in range(2):
                                        nc.tensor.matmul(
                                            out=ps2f[64 * hf : 64 * hf + 64,
                                                     64 * p : 64 * p + 64],
                                            lhsT=sb1[:, ri * 256 + p * 64 :
                                                     ri * 256 + p * 64 + 64],
                                            rhs=dmb[:, _D2_O + ri * 128
                                                    + hf * 64 : _D2_O
                                                    + ri * 128 + hf * 64 + 64],
                                            start=(ri == 0),
                                            stop=(ri == 1),
                                            tile_position=(0, 64 * hf),
                                        )
                            copy(cp_eng(s2_eng),
                                 out=sbov[:, pg],
                                 in_=ps2f.rearrange(
                                     "m (p sl f2) -> m sl p f2",
                                     p=4, sl=2, f2=32))

                if not probe_no_dma:
                    for half in range(2):
                        nc.sync.dma_start(
                            # channel c = pg*16 + sl*8 + gp:
                            # (pg sl gp f1q p f2) -> (gp f1q pg sl p f2)
                            # merges to [(gp f1q)=64, (pg sl)=16, (p f2)=128]
                            out=y[b, 128 * half : 128 * half + 128].rearrange(
                                "(pg sl gp) (f1q p) f2 -> pg sl gp f1q p f2",
                                pg=8, sl=2, gp=8, f1q=8, p=4,
                            ).transpose([2, 3, 0, 1, 4, 5]),
                            in_=sbo[64 * half : 64 * half + 64, :],
                        )
    if split_waits:
        _split_multi_waits(nc)
    return nc


_CACHED = {}


def _get_program():
    if "nc" not in _CACHED:
        _CACHED["nc"] = build_program()
        _CACHED["consts"] = _dft_constants()
    return _CACHED["nc"], _CACHED["consts"]


def kernel(x: np.ndarray) -> np.ndarray:
    assert x.shape == (B_FULL, C2, H, W) and x.dtype == np.float32
    nc, dmats = _get_program()
    x = np.ascontiguousarray(x)
    in_maps = [
        {"x": x[BPC * k : BPC * (k + 1)], "dmats": dmats}
        for k in range(N_CORES)
    ]
    res = run_bass_kernel_spmd(nc, in_maps, list(range(N_CORES)))
    out = np.concatenate(
        [res.results[k]["y"] for k in range(N_CORES)], axis=0
    )
    return out.astype(np.float32, copy=False)


if __name__ == "__main__":
    rng = np.random.default_rng(0)
    x = rng.standard_normal((B_FULL, C2, H, W)).astype(np.float32)
    y = kernel(x)
    print("kernel output", y.shape, y.dtype)


# revision 28
# speedup vs baseline: 1.3568x; 1.1457x over previous
"""Spectral pooling (FFT2 -> crop low freqs -> IFFT2) as dense DFT matmuls on TRN2.

Input  x: (32, 256, 64, 64) fp32  -- channels 0:128 real part, 128:256 imag part
Output y: (32, 256, 32, 32) fp32

Math: per complex image X (64x64), Y = A @ X @ A.T with
  A = sqrt(1/(64*32)) * IDFT32 @ Crop @ DFT64   (32x64 complex)
Sharding: batch dim across 8 cores (4 batches/core), no communication.

Layout scheme (all DMA access patterns <= 3 dims, HBM-side chunks >= 512B):
  Loads put h-row PAIRS (or QUADS for the first NQ channel-blocks) of each
  channel in one partition line: partition = (par, h//2), free =
  (pg, sl, gab, h%2, w); the channel map c = pg*16 + sl*8 + gab*4 + par makes
  both the load and store APs merge into 3 dims with 512B last-dim chunks,
  and the fp32->bf16 cast rides the SWDGE descriptors. A small on-chip
  permute copy swaps (sl, h%2) so stage-1 matmuls get a contiguous
  128-column stationary operand (the ISA allows only one free dim there).
  Stage 1 (contract h) does 4 accumulating matmuls (Xr/Xi x h-parity) per
  8-channel group; stage 2 (contract w) computes Yr into PSUM partitions
  0:64 and Yi into 64:128 (complex combine via +/- DFT constant halves and
  tile_position), so the single PSUM->SBUF copy per group and the stores
  stay partition-aligned.
"""

import math

import numpy as np

from concourse import bass, mybir
from concourse.bass_utils import run_bass_kernel_spmd
from concourse.tile import TileContext

N_CORES = 8
B_FULL, C2, H, W = 32, 256, 64, 64
HP, WP = 32, 32
BPC = B_FULL // N_CORES  # batches per core

NQ = 2  # channel-blocks (of 16) per batch loaded with h-quad layout

F32 = mybir.dt.float32
BF16 = mybir.dt.bfloat16

# const column layout in dmats [128, 5376]
_D1P_O = 0      # 4 x 256  (kind*2+s), cols (ri2, p4, par4, f1q8)
_D1Q_O = 1024   # 8 x 512  (kind*4+shr), cols (ri2, p4, par8, f1q8)
_D2_O = 5120    # 4 x 64   (half*2+ri), cols (sl2, f2_32)
_NCOL = 5376


def _split_multi_waits(nc):
    """This walrus build rejects instructions carrying more than one semaphore
    wait. Hoist extra waits onto same-engine NOPs inserted just before the
    instruction (engine queues execute in order, so blocking is equivalent)."""
    n_split = 0
    for f in nc.m.functions:
        for bb in f.blocks:
            insts = bb.instructions
            out = []
            for inst in insts:
                si = inst.sync_info
                waits = list(si.on_wait) if si and si.on_wait else []
                if len(waits) > 1:
                    si.on_wait = waits[-1:]
                    for w in waits[:-1]:
                        nop = mybir.InstNoOp(
                            name=nc.get_next_instruction_name(),
                            ins=[],
                            outs=[],
                            engine=inst.engine,
                            sync_info=mybir.SyncInfo(on_wait=[w], on_update=[]),
                        )
                        out.append(nop)
                        n_split += 1
                out.append(inst)
            if len(out) != len(insts):
                insts[:] = out
    return n_split


def _dft_constants():
    """bf16 [128, 5376]: stage-1 pair/quad and stage-2 half constants."""
    topf = int(math.ceil(H * 0.5 / 2))  # 16
    midf = H // 2 + topf  # 48
    F = np.exp(-2j * np.pi * np.outer(np.arange(H), np.arange(H)) / H)
    G = np.exp(2j * np.pi * np.outer(np.arange(HP), np.arange(HP)) / HP)
    keep = list(range(topf)) + list(range(midf, H))
    S = np.zeros((HP, H))
    S[np.arange(HP), keep] = 1
    A = (G @ S @ F) / np.sqrt(H * W * HP * WP) ** 0.5
    Ar = A.real.astype(np.float32)  # [32, 64]
    Ai = A.imag.astype(np.float32)

    # f1 reindex: col-position (p, f1q) holds frequency f1 = 4*f1q + p
    perm = np.empty(32, np.int64)  # perm[p*8 + f1q] = 4*f1q + p
    for p in range(4):
        for f1q in range(8):
            perm[p * 8 + f1q] = 4 * f1q + p
    Arp, Aip = Ar[perm], Ai[perm]

    dm = np.zeros((128, _NCOL), np.float32)
    # D1 pair [(par4,he32), (ri2, p4, par4, f1q8)], value at h = 2*he + s
    for kind in range(2):
        for s in range(2):
            o = _D1P_O + (kind * 2 + s) * 256
            for par in range(4):
                rows = slice(par * 32, par * 32 + 32)  # he
                h = 2 * np.arange(32) + s
                v0, v1 = (Arp, Aip) if kind == 0 else (-Aip, Arp)
                # cols ri*128 + (p*8+f1q=j)*4?? -> col = ri*128 + j4*32??
                for j in range(32):  # j = p*8 + f1q -> col ri*128 + p*32 + par*8 + f1q
                    p, f1q = j // 8, j % 8
                    dm[rows, o + p * 32 + par * 8 + f1q] = v0[j][h]
                    dm[rows, o + 128 + p * 32 + par * 8 + f1q] = v1[j][h]
    # D1 quad [(par8,hq16), (ri2, p4, par8, f1q8)], value at h = 4*hq + shr
    for kind in range(2):
        for shr in range(4):
            o = _D1Q_O + (kind * 4 + shr) * 512
            for par in range(8):
                rows = slice(par * 16, par * 16 + 16)  # hq
                h = 4 * np.arange(16) + shr
                v0, v1 = (Arp, Aip) if kind == 0 else (-Aip, Arp)
                for j in range(32):
                    p, f1q = j // 8, j % 8
                    dm[rows, o + p * 64 + par * 8 + f1q] = v0[j][h]
                    dm[rows, o + 256 + p * 64 + par * 8 + f1q] = v1[j][h]
    # D2 per ri: [(sl2,w64), (hf2, sl2, f2_32)=128]
    for ri in range(2):
        for half in range(2):
            o = _D2_O + ri * 128 + half * 64
            v = (Ar, -Ai)[ri] if half == 0 else (Ai, Ar)[ri]
            for sl in range(2):
                dm[sl * 64 : sl * 64 + 64, o + sl * 32 : o + sl * 32 + 32] = v.T
    return dm.astype(mybir.dt.np(BF16))


def build_program(reps: int = 1, nq: int = NQ, split_waits: bool = True,
                  probe_no_compute: bool = False,
                  probe_no_dma: bool = False,
                  ib: int = 2, s1b: int = 8, p1b: int = 4, p2b: int = 4,
                  ob: int = 3, s2_mode: str = "tp16",
                  perm_eng: str = "v", s1_eng: str = "alt", s2_eng: str = "a",
                  big_perm: bool = False):
    """reps > 1 unrolls the whole pipeline in-NEFF over the same data so the
    marginal cost per rep can be measured without the ~65ms axon dispatch
    overhead."""
    npair = 8 - nq
    nc = bass.Bass("TRN2", target_bir_lowering=False, debug=False)
    x = nc.dram_tensor("x", [BPC, C2, H, W], F32, kind="ExternalInput").ap()
    dm = nc.dram_tensor("dmats", [128, _NCOL], BF16, kind="ExternalInput").ap()
    y = nc.dram_tensor("y", [BPC, C2, HP, WP], F32, kind="ExternalOutput").ap()

    with TileContext(nc) as tc:
        with (
            tc.tile_pool(name="consts", bufs=1) as cpool,
            tc.tile_pool(name="inp", bufs=ib) as ipool,
            tc.tile_pool(name="sb1", bufs=s1b) as s1pool,
            tc.tile_pool(name="sbout", bufs=ob) as opool,
            tc.tile_pool(name="ps1", bufs=p1b, space="PSUM") as p1pool,
            tc.tile_pool(name="ps2", bufs=p2b, space="PSUM") as p2pool,
        ):
            dmb = cpool.tile([128, _NCOL], BF16, tag="dmb")
            nc.sync.dma_start(out=dmb, in_=dm)
            d1p = [dmb[:, _D1P_O + j * 256 : _D1P_O + (j + 1) * 256]
                   for j in range(4)]
            d1q = [dmb[:, _D1Q_O + j * 512 : _D1Q_O + (j + 1) * 512]
                   for j in range(8)]
            d2 = [dmb[:, _D2_O + j * 128 : _D2_O + (j + 1) * 128]
                  for j in range(2)]

            def in_tiles(pfx):
                t = {}
                if npair:
                    t["pr"] = ipool.tile([128, npair * 512], BF16,
                                         tag=pfx + "pr", name=pfx + "pr")
                    t["pi"] = ipool.tile([128, npair * 512], BF16,
                                         tag=pfx + "pi", name=pfx + "pi")
                if nq:
                    t["qr"] = ipool.tile([128, nq * 512], BF16,
                                         tag=pfx + "qr", name=pfx + "qr")
                    t["qi"] = ipool.tile([128, nq * 512], BF16,
                                         tag=pfx + "qi", name=pfx + "qi")
                return t

            cp_cnt = [0]

            def cp_eng(which):
                if which == "alt":
                    cp_cnt[0] += 1
                    which = "va"[cp_cnt[0] % 2]
                return {"v": nc.vector, "a": nc.scalar, "p": nc.gpsimd}[which]

            def copy(eng, out, in_):
                if eng is nc.scalar:
                    eng.copy(out=out, in_=in_)
                else:
                    eng.tensor_copy(out=out, in_=in_)

            if probe_no_dma:
                raw_fixed = in_tiles("r")
                for t in raw_fixed.values():
                    nc.vector.memset(t, 0.0)
                sbo_fixed = opool.tile([128, 2048], F32, tag="sbo", name="sbo")

            for b in [b for _ in range(reps) for b in range(BPC)]:
                if probe_no_dma:
                    raw = raw_fixed
                else:
                    raw = in_tiles("r")
                    for kind, key in ((0, "pr"), (1, "pi")):
                        half = x[b, 128 * kind : 128 * kind + 128]
                        if npair:
                            # split so compute can start after the first half
                            cuts = [0, npair // 2, npair]
                            for c0, c1 in zip(cuts, cuts[1:]):
                                if c0 == c1:
                                    continue
                                nc.gpsimd.dma_start(
                                    out=raw[key][:, c0 * 512 : c1 * 512],
                                    # channel c = pg*16 + sl*8 + gab*4 + par:
                                    # (pg sl gab par he s w) ->
                                    # (par he pg sl gab s w); merges to
                                    # [(par he)=128, (pg sl gab), (s w)]
                                    in_=half[(nq + c0) * 16 : (nq + c1) * 16]
                                    .rearrange(
                                        "(pg sl gab par) (he s) w"
                                        " -> pg sl gab par he s w",
                                        pg=c1 - c0, sl=2, gab=2, par=4,
                                        he=32, s=2,
                                    ).transpose([3, 4, 0, 1, 2, 5, 6]),
                                )
                        if nq:
                            nc.gpsimd.dma_start(
                                out=raw["qr" if kind == 0 else "qi"],
                                # channel c = pg*16 + sl*8 + par:
                                # (pg sl par hq s w) -> (par hq pg sl s w)
                                in_=half[: nq * 16].rearrange(
                                    "(pg sl par) (hq s) w"
                                    " -> pg sl par hq s w",
                                    pg=nq, sl=2, par=8, hq=16, s=4,
                                ).transpose([2, 3, 0, 1, 4, 5]),
                            )

                if probe_no_dma:
                    sbo = sbo_fixed
                else:
                    sbo = opool.tile([128, 2048], F32, tag="sbo", name="sbo")
                if probe_no_compute:
                    nc.vector.memset(sbo, 0.0)
                else:
                    # permute (sl, s) -> (s, sl) so stage-1 lhsT is contiguous
                    perm = in_tiles("m")
                    for kind, rk, mk in ((0, "pr", "pr"), (1, "pi", "pi")):
                        if npair:
                            rv = raw[rk].rearrange(
                                "k (pg sl gab s w) -> k pg sl gab s w",
                                pg=npair, sl=2, gab=2, s=2, w=64)
                            for pg in range(npair):
                                if big_perm:
                                    # one [128, 512] copy (4-dim src AP)
                                    copy(cp_eng(perm_eng),
                                         out=perm[mk][:, pg * 512 :
                                                      pg * 512 + 512],
                                         in_=rv[:, pg]
                                         .transpose([0, 2, 3, 1, 4]))
                                    continue
                                for gab in range(2):
                                    copy(cp_eng(perm_eng),
                                         out=perm[mk][:, pg * 512 + gab * 256 :
                                                      pg * 512 + gab * 256 + 256],
                                         in_=rv[:, pg, :, gab, :, :]
                                         .transpose([0, 2, 1, 3]))
                        if nq:
                            qk = "qr" if kind == 0 else "qi"
                            rv = raw[qk].rearrange(
                                "k (pg sl s w) -> k pg sl s w",
                                pg=nq, sl=2, s=4, w=64)
                            for pg in range(nq):
                                copy(cp_eng(perm_eng),
                                     out=perm[qk][:, pg * 512 : pg * 512 + 512],
                                     in_=rv[:, pg, :, :, :]
                                     .transpose([0, 2, 1, 3]))

                    for pg in range(8):
                        sb1 = s1pool.tile([128, 512], BF16, tag="sb1",
                                          name="sb1")
                        if pg < nq:  # h-quad block: perm layout (pg, s4, sl, w)
                            ps1 = p1pool.tile([128, 512], F32, tag="ps1",
                                              name="ps1q")
                            for j, (kind, shr) in enumerate(
                                    (k, s) for k in range(2) for s in range(4)):
                                t = perm["qr" if kind == 0 else "qi"]
                                o = pg * 512 + shr * 128
                                nc.tensor.matmul(
                                    out=ps1,
                                    lhsT=t[:, o : o + 128],
                                    rhs=d1q[kind * 4 + shr],
                                    start=(j == 0),
                                    stop=(j == 7),
                                )
                            copy(cp_eng(s1_eng), out=sb1, in_=ps1)
                        else:  # h-pair block: perm layout (pg, gab, s, sl, w)
                            pgp = pg - nq
                            for gab in range(2):
                                ps1f = p1pool.tile([128, 512], F32, tag="ps1",
                                                   name="ps1p")
                                ps1 = ps1f[:, 0:256]
                                for j, (kind, s) in enumerate(
                                        (k, s) for k in range(2) for s in range(2)):
                                    t = perm["pr" if kind == 0 else "pi"]
                                    o = pgp * 512 + gab * 256 + s * 128
                                    nc.tensor.matmul(
                                        out=ps1,
                                        lhsT=t[:, o : o + 128],
                                        rhs=d1p[kind * 2 + s],
                                        start=(j == 0),
                                        stop=(j == 3),
                                    )
                                # scatter cols (ri, p, pf32) -> sb1 (ri, p, gab, pf32)
                                copy(cp_eng(s1_eng),
                                     out=sb1.rearrange(
                                         "k (ri p gab pf) -> k ri p gab pf",
                                         ri=2, p=4, gab=2, pf=32)[:, :, :, gab, :],
                                     in_=ps1.rearrange(
                                         "k (ri p pf) -> k ri p pf",
                                         ri=2, p=4, pf=32))

                        sbov = sbo.rearrange(
                            "m (pg sl p f2) -> m pg sl p f2",
                            pg=8, sl=2, p=4, f2=32)
                        if s2_mode == "xb8":
                            ps2f = p2pool.tile([128, 512], F32, tag="ps2",
                                               name="ps2")
                            ps2 = ps2f[0:64, :]
                            for p in range(4):
                                for ri in range(2):
                                    nc.tensor.matmul(
                                        out=ps2[:, 128 * p : 128 * p + 128],
                                        lhsT=sb1[:, ri * 256 + p * 64 :
                                                 ri * 256 + p * 64 + 64],
                                        rhs=d2[ri],
                                        start=(ri == 0),
                                        stop=(ri == 1),
                                    )
                            # ps2 cols (p4, hf2, sl2, f2_32); Yi -> sbo[64:128]
                            # via cross-partition-base copy (verified legal)
                            ps2v = ps2.rearrange(
                                "m (p hf sl f2) -> m hf sl p f2",
                                p=4, hf=2, sl=2, f2=32)
                            for hf in range(2):
                                copy(cp_eng(s2_eng),
                                     out=sbov[64 * hf : 64 * hf + 64, pg],
                                     in_=ps2v[:, hf])
                        else:  # tp16: tile_position halves, single copy
                            ps2f = p2pool.tile([128, 256], F32, tag="ps2",
                                               name="ps2")
                            for p in range(4):
                                for hf in range(2):
                                    for ri in range(2):
                                        nc.tensor.matmul(
                                            out=ps2f[64 * hf : 64 * hf + 64,
                                                     64 * p : 64 * p + 64],
                                            lhsT=sb1[:, ri * 256 + p * 64 :
                                                     ri * 256 + p * 64 + 64],
                                            rhs=dmb[:, _D2_O + ri * 128
                                                    + hf * 64 : _D2_O
                                                    + ri * 128 + hf * 64 + 64],
                                            start=(ri == 0),
                                            stop=(ri == 1),
                                            tile_position=(0, 64 * hf),
                                        )
                            copy(cp_eng(s2_eng),
                                 out=sbov[:, pg],
                                 in_=ps2f.rearrange(
                                     "m (p sl f2) -> m sl p f2",
                                     p=4, sl=2, f2=32))

                if not probe_no_dma:
                    for half in range(2):
                        nc.sync.dma_start(
                            # channel c = pg*16 + sl*8 + gp:
                            # (pg sl gp f1q p f2) -> (gp f1q pg sl p f2)
                            # merges to [(gp f1q)=64, (pg sl)=16, (p f2)=128]
                            out=y[b, 128 * half : 128 * half + 128].rearrange(
                                "(pg sl gp) (f1q p) f2 -> pg sl gp f1q p f2",
                                pg=8, sl=2, gp=8, f1q=8, p=4,
                            ).transpose([2, 3, 0, 1, 4, 5]),
                            in_=sbo[64 * half : 64 * half + 64, :],
                        )
    if split_waits:
        _split_multi_waits(nc)
    return nc


_CACHED = {}


def _get_program():
    if "nc" not in _CACHED:
        _CACHED["nc"] = build_program()
        _CACHED["consts"] = _dft_constants()
    return _CACHED["nc"], _CACHED["consts"]


def kernel(x: np.ndarray) -> np.ndarray:
    assert x.shape == (B_FULL, C2, H, W) and x.dtype == np.float32
    nc, dmats = _get_program()
    x = np.ascontiguousarray(x)
    in_maps = [
        {"x": x[BPC * k : BPC * (k + 1)], "dmats": dmats}
        for k in range(N_CORES)
    ]
    res = run_bass_kernel_spmd(nc, in_maps, list(range(N_CORES)))
    out = np.concatenate(
        [res.results[k]["y"] for k in range(N_CORES)], axis=0
    )
    return out.astype(np.float32, copy=False)


if __name__ == "__main__":
    rng = np.random.default_rng(0)
    x = rng.standard_normal((B_FULL, C2, H, W)).astype(np.float32)
    y = kernel(x)
    print("kernel output", y.shape, y.dtype)
